# revision 12
# baseline (speedup 1.0000x reference)
import numpy as np
import ml_dtypes
from contextlib import ExitStack

import concourse.bass as bass
import concourse.bacc as bacc
import concourse.tile as tile
from concourse import mybir
from concourse import bass_utils

K = 7
H = 16
B, S, D = 8, 2048, 1024
L = S - K + 1
C = D // 128
NSB = 4
SB = S // NSB
KH = K * H

F32 = mybir.dt.float32
F32R = mybir.dt.float32r
BF16 = mybir.dt.bfloat16

MM_DT = F32R


def _mm(ap):
    if MM_DT == F32:
        return ap
    return ap.bitcast(MM_DT)


def _host_constants():
    ident = np.eye(128, dtype=np.float32)
    identb = np.eye(128).astype(ml_dtypes.bfloat16)
    h = np.arange(KH) % H
    selsum = ((h[:, None] == h[None, :]) * float(K)).astype(ml_dtypes.bfloat16)
    selk = np.zeros((KH, K * 128), dtype=ml_dtypes.bfloat16)
    for k in range(K):
        for p in range(128):
            selk[16 * k + p % 16, k * 128 + p] = 1.0
    return ident, identb, selsum, selk


def build_program():
    nc = bacc.Bacc(
        "TRN2", target_bir_lowering=False, debug=False, enable_asserts=True
    )

    x_d = nc.dram_tensor("x", [S, D], F32, kind="ExternalInput").ap()
    w_d = nc.dram_tensor("W", [D, KH], BF16, kind="ExternalInput").ap()
    b_d = nc.dram_tensor("b", [KH], F32, kind="ExternalInput").ap()
    ident_d = nc.dram_tensor("ident", [128, 128], F32, kind="ExternalInput").ap()
    identb_d = nc.dram_tensor("identb", [128, 128], BF16, kind="ExternalInput").ap()
    selsum_d = nc.dram_tensor("selsum", [KH, KH], BF16, kind="ExternalInput").ap()
    selk_d = nc.dram_tensor("selk", [KH, K * 128], BF16, kind="ExternalInput").ap()
    out_d = nc.dram_tensor("out", [L, D], F32, kind="ExternalOutput").ap()

    with tile.TileContext(nc) as tc, ExitStack() as ctx:
        singles = ctx.enter_context(tc.tile_pool(name="singles", bufs=1))
        xn_pool = ctx.enter_context(tc.tile_pool(name="xn", bufs=2))
        prod_pool = ctx.enter_context(tc.tile_pool(name="prod", bufs=5))
        outn_pool = ctx.enter_context(tc.tile_pool(name="outn", bufs=3))

        p_tp = ctx.enter_context(tc.tile_pool(name="ptp", bufs=2, space="PSUM"))
        p_log = ctx.enter_context(tc.tile_pool(name="plog", bufs=2, space="PSUM"))
        p_sum = ctx.enter_context(tc.tile_pool(name="psumk", bufs=1, space="PSUM"))
        p_mk = ctx.enter_context(tc.tile_pool(name="pmk", bufs=2, space="PSUM"))
        p_otp = ctx.enter_context(tc.tile_pool(name="potp", bufs=1, space="PSUM"))

        wt = singles.tile([128, C, KH], BF16)
        nc.sync.dma_start(out=wt, in_=w_d.rearrange("(c p) n -> p c n", p=128))
        bias_t = singles.tile([KH, 1], F32)
        nc.sync.dma_start(out=bias_t, in_=b_d.rearrange("(p one) -> p one", one=1))
        ident_t = singles.tile([128, 128], F32)
        nc.sync.dma_start(out=ident_t, in_=ident_d)
        identb_t = singles.tile([128, 128], BF16)
        nc.sync.dma_start(out=identb_t, in_=identb_d)
        selsum_t = singles.tile([KH, KH], BF16)
        nc.sync.dma_start(out=selsum_t, in_=selsum_d)
        selk_t = singles.tile([KH, K, 128], BF16)
        nc.sync.dma_start(
            out=selk_t, in_=selk_d.rearrange("c (k p) -> c k p", k=K)
        )

        xtb = singles.tile([128, C, S], BF16)
        e_full = singles.tile([KH, S], BF16)
        rinv = singles.tile([KH, S], F32)
        en = singles.tile([KH, S], BF16)
        m_all = singles.tile([128, K, S], BF16)
        acc_all = singles.tile([128, C, S], BF16)

        for sb in range(NSB):
            xn = xn_pool.tile([128, 4, D], F32, tag="xn")
            nc.sync.dma_start(
                out=xn,
                in_=x_d[SB * sb : SB * (sb + 1), :].rearrange(
                    "(t p) d -> p t d", p=128
                ),
            )
            for c in range(C):
                ptp = p_tp.tile([128, SB], F32, tag="ptp")
                for tt in range(4):
                    nc.tensor.transpose(
                        ptp[:, 128 * tt : 128 * (tt + 1)],
                        xn[:, tt, 128 * c : 128 * (c + 1)],
                        ident_t,
                    )
                nc.scalar.copy(xtb[:, c, SB * sb : SB * (sb + 1)], ptp)
            plog = p_log.tile([KH, SB], F32, tag="plog")
            for c in range(C):
                nc.tensor.matmul(
                    plog,
                    wt[:, c, :],
                    xtb[:, c, SB * sb : SB * (sb + 1)],
                    start=(c == 0),
                    stop=(c == C - 1),
                )
            nc.scalar.activation(
                e_full[:, SB * sb : SB * (sb + 1)],
                plog,
                mybir.ActivationFunctionType.Exp,
                bias=bias_t,
                scale=1.0,
            )

        for sb in range(NSB):
            sl = slice(SB * sb, SB * (sb + 1))
            psum = p_sum.tile([KH, SB], F32, tag="psumk")
            nc.tensor.matmul(
                psum, selsum_t, e_full[:, sl], start=True, stop=True
            )
            nc.vector.reciprocal(rinv[:, sl], psum)
            nc.gpsimd.tensor_mul(en[:, sl], e_full[:, sl], rinv[:, sl])

        for k in range(K):
            for sb in range(NSB):
                l0 = SB * sb
                nl = min(SB, L - l0)
                pmk = p_mk.tile([128, SB], F32, tag="pmk")
                nc.tensor.matmul(
                    pmk[:, :nl],
                    selk_t[:, k, :],
                    en[:, l0 + K - 1 : l0 + K - 1 + nl],
                    start=True,
                    stop=True,
                )
                nc.scalar.copy(m_all[:, k, l0 : l0 + nl], pmk[:, :nl])

        def _prod(c, k):
            p = prod_pool.tile([128, L], BF16, tag="prod")
            nc.vector.tensor_mul(p, m_all[:, k, :L], xtb[:, c, k : k + L])
            return p

        for c in range(C):
            p0 = _prod(c, 0)
            p1 = _prod(c, 1)
            a01 = prod_pool.tile([128, L], BF16, tag="prod")
            nc.vector.tensor_add(a01, p0, p1)
            p2 = _prod(c, 2)
            p3 = _prod(c, 3)
            a23 = prod_pool.tile([128, L], BF16, tag="prod")
            nc.vector.tensor_add(a23, p2, p3)
            p4 = _prod(c, 4)
            p5 = _prod(c, 5)
            a45 = prod_pool.tile([128, L], BF16, tag="prod")
            nc.vector.tensor_add(a45, p4, p5)
            b0 = prod_pool.tile([128, L], BF16, tag="prod")
            nc.vector.tensor_add(b0, a01, a23)
            p6 = _prod(c, 6)
            b1 = prod_pool.tile([128, L], BF16, tag="prod")
            nc.vector.tensor_add(b1, a45, p6)
            nc.vector.tensor_add(acc_all[:, c, :L], b0, b1)

        NLB = (L + 127) // 128
        for lb in range(NLB):
            l0 = 128 * lb
            nl = min(128, L - l0)
            outn = outn_pool.tile([128, D], F32, tag="outn")
            for half in range(2):
                potp = p_otp.tile([128, 512], BF16, tag="potp")
                for cc in range(4):
                    c = 4 * half + cc
                    nc.tensor.transpose(
                        potp[:nl, 128 * cc : 128 * (cc + 1)],
                        acc_all[:, c, l0 : l0 + nl],
                        identb_t,
                    )
                nc.scalar.copy(outn[:nl, 512 * half : 512 * (half + 1)], potp[:nl, :])
            nc.sync.dma_start(out=out_d[l0 : l0 + nl, :], in_=outn[:nl, :])

    nc.compile()
    return nc


_CACHE = {}


def _get_program():
    if "nc" not in _CACHE:
        _CACHE["nc"] = build_program()
    return _CACHE["nc"]


def kernel(x, W, b):
    x = np.asarray(x, dtype=np.float32)
    W = np.asarray(W, dtype=np.float32).astype(ml_dtypes.bfloat16)
    b = np.asarray(b, dtype=np.float32)
    assert x.shape == (B, S, D), x.shape

    nc = _get_program()
    ident, identb, selsum, selk = _host_constants()
    in_maps = []
    for core in range(B):
        in_maps.append(
            {
                "x": np.ascontiguousarray(x[core]),
                "W": W,
                "b": b,
                "ident": ident,
                "identb": identb,
                "selsum": selsum,
                "selk": selk,
            }
        )
    res = bass_utils.run_bass_kernel_spmd(nc, in_maps, core_ids=list(range(B)))
    out = np.stack([res.results[core]["out"] for core in range(B)], axis=0)
    return out
